# revision 8
# baseline (speedup 1.0000x reference)
"""GPT2 attention (B=2,S=2048,D=2048,H=16) on 8 trn2 NeuronCores.

Sharding: tensor-parallel over heads, 2 heads per core. Each core computes
qkv for its heads, full attention (scores, softmax, attn@v), and a partial
c_proj (its heads' rows of w_proj). Host sums the 8 partial c_proj outputs
and concatenates per-head attention weights.

Layouts (all chosen so no on-device transpose is ever needed):
  - hidden is fed d-major ("ht": [B, D, S]) so QKV matmuls contract d on
    partitions and produce q,k feature-major [hd, t].
  - scores are computed twice: [q,k] orientation for the attn_weights
    output + softmax sums (free-dim reductions), and [k,q] orientation
    (scoresT) to feed attn@v, whose normalization is deferred to the
    c_proj copyback where tokens sit on partitions.
"""

import numpy as np
from contextlib import ExitStack

import concourse.bass as bass
import concourse.bacc as bacc_mod
import concourse.tile as tile
import concourse.mybir as mybir
from concourse.bass_utils import run_bass_kernel_spmd

B, S, D, H, HD = 2, 2048, 2048, 16, 128
P = 128
NCORES = 8
HPC = H // NCORES  # heads per core
f32 = mybir.dt.float32
f32r = mybir.dt.float32r
AF = mybir.ActivationFunctionType
ALU = mybir.AluOpType
AX = mybir.AxisListType

TS = 512          # token slab for QKV
N_TS = S // TS    # 4
DC = D // P       # 16 d-chunks
DH = 8            # d-chunks per hidden half-slab
KC = S // P       # 16 k chunks of 128
QC = S // P       # 16 q chunks of 128
NQS = S // 512    # 4 q slabs of 512


def r(ap):
    return ap


def build_program():
    nc = bacc_mod.Bacc(None, target_bir_lowering=False, debug=False)

    ht = nc.declare_dram_parameter("ht", [B, D, S], f32r, isOutput=False)
    wqk = nc.declare_dram_parameter("wqk", [D, 4 * P], f32r, isOutput=False)
    wv = nc.declare_dram_parameter("wv", [D, HPC * HD], f32r, isOutput=False)
    wp = nc.declare_dram_parameter("wp", [HPC, HD, D], f32r, isOutput=False)
    bqk = nc.declare_dram_parameter("bqk", [P, 4], f32, isOutput=False)
    bv = nc.declare_dram_parameter("bv", [P, HPC * HD], f32, isOutput=False)
    aw = nc.declare_dram_parameter("aw", [B, HPC, S, S], f32, isOutput=True)
    outp = nc.declare_dram_parameter("outp", [B, S, D], f32, isOutput=True)

    with tile.TileContext(nc) as tc, ExitStack() as ctx:
        const = ctx.enter_context(tc.tile_pool(name="const", bufs=1))
        qkp = ctx.enter_context(tc.tile_pool(name="qkp", bufs=1))
        vp = ctx.enter_context(tc.tile_pool(name="vp", bufs=1))
        hidp = ctx.enter_context(tc.tile_pool(name="hidp", bufs=2))
        wtp = ctx.enter_context(tc.tile_pool(name="wtp", bufs=2))
        expp = ctx.enter_context(tc.tile_pool(name="expp", bufs=2))
        atp = ctx.enter_context(tc.tile_pool(name="atp", bufs=1))
        outsp = ctx.enter_context(tc.tile_pool(name="outsp", bufs=2))
        mp = ctx.enter_context(tc.tile_pool(name="mp", bufs=2))
        pp = ctx.enter_context(tc.tile_pool(name="pp", bufs=4))
        stp = ctx.enter_context(tc.tile_pool(name="stp", bufs=2))
        psp = ctx.enter_context(tc.tile_pool(name="psp", bufs=8, space="PSUM"))

        wqk_sb = const.tile([P, DC, 4 * P], f32r)
        nc.sync.dma_start(wqk_sb[:], wqk.rearrange("(dc p) f -> p dc f", p=P))
        wv_sb = const.tile([P, DC, HPC * HD], f32r)
        nc.sync.dma_start(wv_sb[:], wv.rearrange("(dc p) f -> p dc f", p=P))
        wp_sb = const.tile([P, HPC, D], f32r)
        nc.sync.dma_start(wp_sb[:], wp.rearrange("h p d -> p h d"))
        bqk_sb = const.tile([P, 4], f32)
        nc.sync.dma_start(bqk_sb[:], bqk[:, :])
        bv_sb = const.tile([P, HPC * HD], f32)
        nc.sync.dma_start(bv_sb[:], bv[:, :])

        for b in range(B):
            qk_sb = qkp.tile([P, 4, S], f32r)   # [hd, (q0,q1,k0,k1), t]
            v_sb = vp.tile([P, KC, HPC * HD], f32r)  # [t%128, t//128, hd2]

            # ---- QKV projection ----
            for ts_ in range(N_TS):
                qk_ps = [psp.tile([P, TS], f32, tag="bank", name=f"qkps{i}") for i in range(4)]
                v_ps = [psp.tile([P, TS], f32, tag="bank", name=f"vps{i}") for i in range(4)]
                for half in range(2):
                    hsl = hidp.tile([P, DH, TS], f32r)
                    nc.sync.dma_start(
                        hsl[:],
                        ht[b, half * DH * P:(half + 1) * DH * P,
                           ts_ * TS:(ts_ + 1) * TS].rearrange(
                               "(dc p) t -> p dc t", p=P),
                    )
                    for dcl in range(DH):
                        dc = half * DH + dcl
                        first = dc == 0
                        last = dc == DC - 1
                        for fc in range(4):
                            nc.tensor.matmul(
                                qk_ps[fc][:],
                                r(wqk_sb[:, dc, fc * P:(fc + 1) * P]),
                                r(hsl[:, dcl, :]),
                                start=first, stop=last,
                            )
                        for tcl in range(4):
                            nc.tensor.matmul(
                                v_ps[tcl][:, :HPC * HD],
                                r(hsl[:, dcl, tcl * P:(tcl + 1) * P]),
                                r(wv_sb[:, dc, :]),
                                start=first, stop=last,
                            )
                for fc in range(4):
                    nc.scalar.activation(
                        qk_sb[:, fc, ts_ * TS:(ts_ + 1) * TS], qk_ps[fc][:],
                        AF.Identity, bias=bqk_sb[:, fc:fc + 1], scale=1.0,
                    )
                for tcl in range(4):
                    nc.vector.tensor_add(
                        v_sb[:, ts_ * 4 + tcl, :],
                        v_ps[tcl][:, :HPC * HD], bv_sb[:],
                    )

            recips = stp.tile([P, HPC, QC], f32, tag="recips")
            attnT = atp.tile([P, HPC, S], f32r, tag="attnT")
            for h in range(HPC):
                # ---- phase 1: scores [q,k], softmax, attn_weights out ----
                sums = stp.tile([P, QC], f32, tag="sums")
                for qc in range(QC):
                    part = pp.tile([P, 4], f32, tag="part")
                    wt = wtp.tile([P, S], f32)
                    for kg in range(4):
                        sc_ps = psp.tile([P, TS], f32, tag="bank")
                        nc.tensor.matmul(
                            sc_ps[:],
                            r(qk_sb[:, h, qc * P:(qc + 1) * P]),
                            r(qk_sb[:, 2 + h, kg * TS:(kg + 1) * TS]),
                            start=True, stop=True,
                        )
                        nc.scalar.activation(
                            wt[:, kg * TS:(kg + 1) * TS], sc_ps[:],
                            AF.Exp, accum_out=part[:, kg:kg + 1],
                        )
                    nc.vector.tensor_reduce(
                        sums[:, qc:qc + 1], part[:], axis=AX.X, op=ALU.add)
                    nc.vector.reciprocal(
                        recips[:, h, qc:qc + 1], sums[:, qc:qc + 1])
                    nc.vector.tensor_scalar_mul(
                        wt[:], wt[:], recips[:, h, qc:qc + 1])
                    nc.sync.dma_start(
                        aw[b, h, qc * P:(qc + 1) * P, :], wt[:])

                # ---- phase 2/3: scoresT [k,q], exp, attn@v ----
                for qs in range(NQS):
                    av_ps = psp.tile([P, 512], f32, tag="bank")
                    for kc in range(KC):
                        st_ps = psp.tile([P, 512], f32, tag="bank")
                        nc.tensor.matmul(
                            st_ps[:],
                            r(qk_sb[:, 2 + h, kc * P:(kc + 1) * P]),
                            r(qk_sb[:, h, qs * 512:(qs + 1) * 512]),
                            start=True, stop=True,
                        )
                        et = expp.tile([P, 512], f32r)
                        nc.scalar.activation(et[:], st_ps[:], AF.Exp)
                        nc.tensor.matmul(
                            av_ps[:],
                            r(v_sb[:, kc, h * HD:(h + 1) * HD]),
                            r(et[:]),
                            start=(kc == 0), stop=(kc == KC - 1),
                        )
                    nc.vector.tensor_copy(
                        attnT[:, h, qs * 512:(qs + 1) * 512], av_ps[:])

            # ---- c_proj partial: out[t, dout] for this core's heads ----
            for tc_ in range(QC):
                osb = outsp.tile([P, D], f32)
                for ng in range(4):
                    p1 = psp.tile([P, 512], f32, tag="bank")
                    nc.tensor.matmul(
                        p1[:],
                        r(attnT[:, 0, tc_ * P:(tc_ + 1) * P]),
                        r(wp_sb[:, 0, ng * 512:(ng + 1) * 512]),
                        start=True, stop=True,
                    )
                    p2 = psp.tile([P, 512], f32, tag="bank")
                    nc.tensor.matmul(
                        p2[:],
                        r(attnT[:, 1, tc_ * P:(tc_ + 1) * P]),
                        r(wp_sb[:, 1, ng * 512:(ng + 1) * 512]),
                        start=True, stop=True,
                    )
                    m1 = mp.tile([P, 512], f32, tag="m1")
                    nc.scalar.activation(
                        m1[:], p1[:], AF.Copy, bias=0.0,
                        scale=recips[:, 0, tc_:tc_ + 1],
                    )
                    m2 = mp.tile([P, 512], f32, tag="m2")
                    nc.vector.tensor_scalar_mul(
                        m2[:], p2[:], recips[:, 1, tc_:tc_ + 1])
                    nc.vector.tensor_add(
                        osb[:, ng * 512:(ng + 1) * 512], m1[:], m2[:])
                nc.sync.dma_start(
                    outp[b, tc_ * P:(tc_ + 1) * P, :], osb[:])

    nc.finalize()
    return nc


_CACHE = {}


def kernel(hidden_states, w_attn, b_attn, w_proj, b_proj):
    hidden_states = np.asarray(hidden_states, dtype=np.float32)
    w_attn = np.asarray(w_attn, dtype=np.float32)
    b_attn = np.asarray(b_attn, dtype=np.float32)
    w_proj = np.asarray(w_proj, dtype=np.float32)
    b_proj = np.asarray(b_proj, dtype=np.float32)

    if "nc" not in _CACHE:
        _CACHE["nc"] = build_program()
    nc = _CACHE["nc"]

    s = 1.0 / np.sqrt(np.float32(HD))
    ht = np.ascontiguousarray(hidden_states.transpose(0, 2, 1))
    wq = w_attn[:, :D]
    wk = w_attn[:, D:2 * D]
    wv_full = w_attn[:, 2 * D:]
    bq = b_attn[:D]
    bk = b_attn[D:2 * D]
    bv_full = b_attn[2 * D:]

    in_maps = []
    for c in range(NCORES):
        g0 = c * HPC
        cols = slice(g0 * HD, (g0 + HPC) * HD)
        wqk_c = np.concatenate(
            [wq[:, g0 * HD:(g0 + 1) * HD] * s,
             wq[:, (g0 + 1) * HD:(g0 + 2) * HD] * s,
             wk[:, g0 * HD:(g0 + 1) * HD],
             wk[:, (g0 + 1) * HD:(g0 + 2) * HD]], axis=1)
        bqk_c = np.stack(
            [bq[g0 * HD:(g0 + 1) * HD] * s,
             bq[(g0 + 1) * HD:(g0 + 2) * HD] * s,
             bk[g0 * HD:(g0 + 1) * HD],
             bk[(g0 + 1) * HD:(g0 + 2) * HD]], axis=1)
        wv_c = wv_full[:, cols]
        bv_c = np.tile(bv_full[cols][None, :], (P, 1))
        wp_c = w_proj[cols, :].reshape(HPC, HD, D)
        in_maps.append({
            "ht": ht,
            "wqk": np.ascontiguousarray(wqk_c),
            "wv": np.ascontiguousarray(wv_c),
            "wp": np.ascontiguousarray(wp_c),
            "bqk": np.ascontiguousarray(bqk_c),
            "bv": np.ascontiguousarray(bv_c),
        })

    res = run_bass_kernel_spmd(nc, in_maps, list(range(NCORES))).results

    attn_weights = np.empty((B, H, S, S), dtype=np.float32)
    attn_output = np.zeros((B, S, D), dtype=np.float32)
    for c in range(NCORES):
        g0 = c * HPC
        attn_weights[:, g0:g0 + HPC] = res[c]["aw"]
        attn_output += res[c]["outp"]
    attn_output += b_proj
    return attn_output, attn_weights
